# revision 1
# baseline (speedup 1.0000x reference)
"""Multi-head attention (B=4, S=2048, D=1024, H=16) on 8 TRN2 NeuronCores.

Sharding: core c -> (batch b = c//2, head-group g = c%2). Each core computes
8 heads for one batch: QKV projections restricted to its 512 output dims,
attention for its heads, and a partial output projection over its 512
contraction dims of W_o. Host sums the two partial outputs per batch.

Device layout (per core, all matmul operands bf16, PSUM fp32):
  inputs  xqT/xkT/xvT [1024, S]   (X^T: feature dim on partitions)
          wqT/wkT/wvT [1024, 512] (W.T slices; 1/sqrt(dk) folded into wqT)
          woT         [512, 1024] (W_o slice transposed)
  output  outT        [1024, S]   (partial final^T, fp32)

Pipeline: Q^T/K^T projections (out-dim on partitions), V projection (seq on
partitions) with a ones column appended per head; per head: scores^T =
(K_h^T)^T-stationary x Q_h^T-moving so k-positions land on partitions; exp on
ScalarE directly from PSUM; PV with the ones column producing the softmax
denominator Z in column 64; per-partition reciprocal + scale on VectorE; PE
transpose of the normalized head output; final projection vs woT.
"""

import numpy as np
import ml_dtypes

B = 4
S = 2048
D = 1024
H_LOCAL = 8          # heads per core
DK = 64
G = H_LOCAL * DK     # 512 output dims per core
N_CORES = 8

_BUILD_CACHE = {}
_BUILD_VERSION = 5   # bump on any device-program change: busts the neuronxcc
                     # cache, whose module hash ignores custom_call contents

bf16 = ml_dtypes.bfloat16


def _build(s=S, debug=False, stage=5, delay_us=0):
    """stage: 1=DMA only, 2=+QKV proj, 3=+QK/exp, 4=+PV/drain, 5=full.
    delay_us: adds a nop chain of that length on SyncE (timing calibration)."""
    import concourse.tile as tile
    from concourse import bacc, mybir
    from concourse.masks import make_identity

    f32 = mybir.dt.float32
    b16 = mybir.dt.bfloat16

    assert s % 512 == 0
    NKT = s // 128          # k-position tiles per head
    NQT = s // 128          # q tiles per head
    NCH = s // 512          # 512-wide chunks of the seq dim
    HALF = min(s, 1024)     # QK psum tile width (<= 2 PSUM banks)
    NH = s // HALF          # halves per seq dim
    NCH_H = HALF // 512     # 512-chunks per half

    nc = bacc.Bacc("TRN2", target_bir_lowering=False, debug=False,
                   num_devices=N_CORES)

    xqT = nc.dram_tensor("xqT", [D, s], b16, kind="ExternalInput")
    xkT = nc.dram_tensor("xkT", [D, s], b16, kind="ExternalInput")
    xvT = nc.dram_tensor("xvT", [D, s], b16, kind="ExternalInput")
    wqT = nc.dram_tensor("wqT", [D, G], b16, kind="ExternalInput")
    wkT = nc.dram_tensor("wkT", [D, G], b16, kind="ExternalInput")
    wvT = nc.dram_tensor("wvT", [D, G], b16, kind="ExternalInput")
    woT = nc.dram_tensor("woT", [G, D], b16, kind="ExternalInput")
    nc.dram_tensor("vtag", [stage, _BUILD_VERSION + delay_us], f32,
                   kind="ExternalInput")
    outT = nc.dram_tensor("outT", [D, s], f32, kind="ExternalOutput")
    if debug:
        qTd = nc.dram_tensor("qTd", [G, s], b16, kind="ExternalOutput")
        kTd = nc.dram_tensor("kTd", [G, s], b16, kind="ExternalOutput")
        vd = nc.dram_tensor("vd", [s, H_LOCAL * (DK + 1)], b16,
                            kind="ExternalOutput")
        attnd = nc.dram_tensor("attnd", [s, s], b16, kind="ExternalOutput")
        aTd = nc.dram_tensor("aTd", [G, s], b16, kind="ExternalOutput")

    with tile.TileContext(nc) as tc:
        with (
            tc.tile_pool(name="w", bufs=1) as wpool,
            tc.tile_pool(name="big", bufs=36) as big,
            tc.tile_pool(name="vp", bufs=NKT) as vpool,
            tc.tile_pool(name="sm", bufs=4) as small,
            tc.tile_pool(name="ps", bufs=1, space="PSUM") as psum,
        ):
            # ---- weights + identity ----
            wq_s = wpool.tile([128, 8, G], b16, tag="wq")
            wk_s = wpool.tile([128, 8, G], b16, tag="wk")
            wv_s = wpool.tile([128, 8, G], b16, tag="wv")
            for t in range(8):
                nc.sync.dma_start(wq_s[:, t, :], wqT[t * 128:(t + 1) * 128, :])
                nc.sync.dma_start(wk_s[:, t, :], wkT[t * 128:(t + 1) * 128, :])
                nc.sync.dma_start(wv_s[:, t, :], wvT[t * 128:(t + 1) * 128, :])
            ident = wpool.tile([128, 128], b16, tag="ident")
            make_identity(nc, ident[:])


            # ablation plumbing: tiny live reads that defeat DCE per stage
            sink_t = wpool.tile([128, 512], f32, tag="sink")
            sink_n = [0]

            def sink(ap):
                c = sink_n[0]
                sink_n[0] += 1
                while len(ap.shape) > 2:
                    ap = ap[:, 0]
                nc.vector.tensor_copy(sink_t[0:1, c:c + 1], ap[0:1, 0:1])

            # ---- X^T inputs ----
            xq_s, xk_s, xv_s = [], [], []
            for src, dst in ((xqT, xq_s), (xkT, xk_s), (xvT, xv_s)):
                for t in range(8):
                    xt = big.tile([128, s], b16, tag="big")
                    nc.sync.dma_start(xt[:], src[t * 128:(t + 1) * 128, :])
                    dst.append(xt)

            if stage == 1:
                for xt in xq_s + xk_s + xv_s:
                    sink(xt)
                for wt in (wq_s, wk_s, wv_s):
                    sink(wt)

            # ---- Q^T / K^T projections: out [G, s], out-dim on partitions ----
            def proj_T(w_s, x_s, out_tiles, o):
                ot = big.tile([128, s], b16, tag="big")
                for half in range(NH):
                    ps = psum.tile([128, HALF], f32, tag="mm", bufs=2)
                    for c in range(NCH_H):
                        cs = slice(half * HALF + c * 512,
                                   half * HALF + (c + 1) * 512)
                        ls = slice(c * 512, (c + 1) * 512)
                        for i in range(8):
                            nc.tensor.matmul(
                                ps[:, ls],
                                lhsT=w_s[:, i, o * 128:(o + 1) * 128],
                                rhs=x_s[i][:, cs],
                                start=(i == 0), stop=(i == 7),
                            )
                    nc.vector.tensor_copy(
                        ot[:, half * HALF:(half + 1) * HALF], ps[:])
                out_tiles.append(ot)

            qT_s, kT_s = [], []

            # ---- V projection (one seq-tile step; interleaved into head 0) ----
            vp_s = []

            def emit_vproj_step(r):
                vt = vpool.tile([128, H_LOCAL, DK + 1], b16, tag="vp")
                nc.vector.memset(vt[:], 1.0)
                ps = psum.tile([128, 512], f32, tag="pv", bufs=4)
                for i in range(8):
                    nc.tensor.matmul(
                        ps[:],
                        lhsT=xv_s[i][:, r * 128:(r + 1) * 128],
                        rhs=wv_s[:, i, :],
                        start=(i == 0), stop=(i == 7),
                    )
                nc.vector.tensor_copy(
                    vt[:, :, 0:DK],
                    ps[:].rearrange("p (h d) -> p h d", h=H_LOCAL),
                )
                if debug:
                    nc.sync.dma_start(
                        vd[r * 128:(r + 1) * 128, :],
                        vt[:].rearrange("p h d -> p (h d)"))
                vp_s.append(vt)

            # ---- attention ----
            aT_s = [big.tile([128, s], b16, tag="big", name=f"aT{i}")
                    for i in range(4)]

            def head_slices(h):
                return h // 2, slice((h % 2) * 64, (h % 2) * 64 + 64)

            attn_tiles = {}   # (h, kt) -> sbuf tile
            pv_ps = {}        # h -> list of PV psum tiles

            def emit_qk(h, kt):
                ti, prt = head_slices(h)
                at = big.tile([128, s], b16, tag="big")
                for half in range(NH):
                    ps = psum.tile([128, HALF], f32, tag="mm", bufs=2)
                    for c in range(NCH_H):
                        cs = slice(half * HALF + c * 512,
                                   half * HALF + (c + 1) * 512)
                        nc.tensor.matmul(
                            ps[:, c * 512:(c + 1) * 512],
                            lhsT=kT_s[ti][prt, kt * 128:(kt + 1) * 128],
                            rhs=qT_s[ti][prt, cs],
                            start=True, stop=True,
                        )
                    nc.scalar.activation(
                        at[:, half * HALF:(half + 1) * HALF], ps[:],
                        func=mybir.ActivationFunctionType.Exp)
                if debug and h == 0:
                    nc.sync.dma_start(attnd[kt * 128:(kt + 1) * 128, :], at[:])
                attn_tiles[(h, kt)] = at

            def emit_pv_step(h, qt):
                pv = psum.tile([128, DK + 1], f32, tag="pv", bufs=4,
                               name=f"pvps{h}_{qt}")
                pv_ps[(h, qt)] = pv
                for kt in range(NKT):
                    nc.tensor.matmul(
                        pv[:],
                        lhsT=attn_tiles[(h, kt)][:, qt * 128:(qt + 1) * 128],
                        rhs=vp_s[kt][:, h, :],
                        start=(kt == 0), stop=(kt == NKT - 1),
                        skip_group_check=True,
                    )
                if qt == NQT - 1:
                    for kt in range(NKT):
                        attn_tiles.pop((h, kt))

            def emit_pv_drain(h, qt):
                ti, prt = head_slices(h)
                pv = pv_ps.pop((h, qt))
                rz = small.tile([128, 1], f32, tag="rz")
                nc.vector.reciprocal(rz[:], pv[:, 64:65])
                a_t = small.tile([128, DK], b16, tag="a")
                nc.vector.tensor_scalar_mul(a_t[:], pv[:, 0:64], rz[:])
                tr = psum.tile([64, 128], b16, tag="pv", bufs=4, name="trp")
                nc.tensor.transpose(tr[:], a_t[:], ident[:])
                nc.vector.tensor_copy(
                    aT_s[ti][prt, qt * 128:(qt + 1) * 128], tr[:])

            if stage == 2:
                for o in range(4):
                    proj_T(wq_s, xq_s, qT_s, o)
                for o in range(4):
                    proj_T(wk_s, xk_s, kT_s, o)
                for r in range(NKT):
                    emit_vproj_step(r)
                for t in qT_s + kT_s + vp_s:
                    sink(t)
            elif stage >= 3:
                for o in range(4):
                    proj_T(wq_s, xq_s, qT_s, o)
                for o in range(4):
                    proj_T(wk_s, xk_s, kT_s, o)
                for h in range(H_LOCAL):
                    for kt in range(NKT):
                        emit_qk(h, kt)
                        if h == 0:
                            emit_vproj_step(kt)
                        if stage == 3:
                            sink(attn_tiles.pop((h, kt)))
                        elif h > 0:
                            emit_pv_step(h - 1, kt)
                            emit_pv_drain(h - 1, kt)
                if stage >= 4:
                    for qt in range(NQT):
                        emit_pv_step(H_LOCAL - 1, qt)
                        emit_pv_drain(H_LOCAL - 1, qt)
                if stage == 3:
                    for t in vp_s:
                        sink(t)
                if stage == 4:
                    for t in aT_s:
                        sink(t)
                if debug:
                    for o in range(4):
                        nc.sync.dma_start(qTd[o * 128:(o + 1) * 128, :],
                                          qT_s[o][:])
                        nc.sync.dma_start(kTd[o * 128:(o + 1) * 128, :],
                                          kT_s[o][:])

            if debug:
                for i in range(4):
                    nc.sync.dma_start(aTd[i * 128:(i + 1) * 128, :], aT_s[i][:])

            if stage < 5:
                fo = small.tile([128, 512], f32, tag="fout", bufs=4)
                nc.vector.tensor_copy(fo[:], sink_t[:])
                nc.sync.dma_start(outT[0:128, 0:512], fo[:])

            # ---- output projection: outT[o*128:, c*512:] = sum_i woT_i.T @ aT_i ----
            if stage >= 5:
                wo_a = big.tile([128, 2, D], b16, tag="big", name="wo_a")
                wo_b = big.tile([128, 2, D], b16, tag="big", name="wo_b")
                for t in range(2):
                    nc.sync.dma_start(wo_a[:, t, :], woT[t * 128:(t + 1) * 128, :])
                    nc.sync.dma_start(wo_b[:, t, :],
                                      woT[(2 + t) * 128:(3 + t) * 128, :])
                wo_v = [wo_a[:, 0, :], wo_a[:, 1, :], wo_b[:, 0, :], wo_b[:, 1, :]]
            for o in range(8 if stage >= 5 else 0):
                for c in range(NCH):
                    cs = slice(c * 512, (c + 1) * 512)
                    ps = psum.tile([128, 512], f32, tag="pv", bufs=4)
                    for i in range(4):
                        nc.tensor.matmul(
                            ps[:],
                            lhsT=wo_v[i][:, o * 128:(o + 1) * 128],
                            rhs=aT_s[i][:, cs],
                            start=(i == 0), stop=(i == 3),
                        )
                    fo = small.tile([128, 512], f32, tag="fout", bufs=4)
                    nc.vector.tensor_copy(fo[:], ps[:])
                    nc.sync.dma_start(outT[o * 128:(o + 1) * 128, cs], fo[:])

    nc.compile()
    return nc


def _host_prep(Q_in, K_in, V_in, W_q, W_k, W_v, W_o, s=S):
    """Build per-core input maps (host-side shard + transpose + bf16 cast)."""
    in_maps = []
    scale = 1.0 / np.sqrt(np.float32(DK))
    for c in range(N_CORES):
        b, g = divmod(c, 2)
        gs = slice(g * G, (g + 1) * G)
        m = {
            "xqT": np.ascontiguousarray(Q_in[b].T).astype(bf16),
            "xkT": np.ascontiguousarray(K_in[b].T).astype(bf16),
            "xvT": np.ascontiguousarray(V_in[b].T).astype(bf16),
            "wqT": np.ascontiguousarray((W_q[gs, :] * scale).T).astype(bf16),
            "wkT": np.ascontiguousarray(W_k[gs, :].T).astype(bf16),
            "wvT": np.ascontiguousarray(W_v[gs, :].T).astype(bf16),
            "woT": np.ascontiguousarray(W_o[:, gs].T).astype(bf16),
            "vtag": np.zeros((5, _BUILD_VERSION), np.float32),
        }
        in_maps.append(m)
    return in_maps


def kernel(Q_in, K_in, V_in, W_q, W_k, W_v, W_o):
    from concourse.bass_utils import run_bass_kernel_spmd

    if "nc" not in _BUILD_CACHE:
        _BUILD_CACHE["nc"] = _build()
    nc = _BUILD_CACHE["nc"]

    in_maps = _host_prep(np.asarray(Q_in, np.float32), np.asarray(K_in, np.float32),
                         np.asarray(V_in, np.float32), np.asarray(W_q, np.float32),
                         np.asarray(W_k, np.float32), np.asarray(W_v, np.float32),
                         np.asarray(W_o, np.float32))
    res = run_bass_kernel_spmd(nc, in_maps, core_ids=list(range(N_CORES)))

    out = np.empty((B, S, D), np.float32)
    for b in range(B):
        acc = res.results[2 * b]["outT"] + res.results[2 * b + 1]["outT"]
        out[b] = acc.T
    return out

